# revision 21
# baseline (speedup 1.0000x reference)
"""DFSMN forward on 8 Trainium2 NeuronCores (Bass/Tile).

Math: the reference computes
    base_t = (1+l0)*v_t + sum_{k=1..10} r_{k-1} v_{t+k}
    p_t    = base_t + sum_{k=1..19} l_k p_{t-k}        (per-feature IIR)
which equals a per-feature convolution  p = g * v  (plus boundary fixes)
with g built from the recurrence impulse response (geometric decay).

The kernel computes the RESIDUAL delta = p - v on device (the identity
tap is removed from the filter table host-side), scales by S=32 and
emits int8; the host adds v back in fp32.  Start-of-sequence boundary
fixes (head restore of v[0:10], r-tap overcount, identity for t<10)
touch only t < 384 and contract with v[0:10], so they are applied on
the HOST in fp64 - the device runs a pure band-matmul chain.

Mixed FIR length: the recurrence tail beyond lag ~119 is negligible for
~97% of features (median L2 7e-6); only features with near-unstable
recurrences need the third Toeplitz block.  The host sorts each core's
64 features by tail norm: the top-8 form the STRAGGLER group (3 blocks,
lags to i+246); the remaining 56 run 2 blocks (lags to i+118; their
truncated tail is <~1.5e-4, invisible next to the 3.3e-3 int8 floor).
This cuts PE time ~40% and band traffic ~30%.

Device mapping (features sharded 8 ways -> 64 per core):
  per feature: 2-3 Toeplitz-band matmuls (block q covers lags
  [i-137+128q, i-10+128q] for output row i) accumulate 32*delta into
  one PSUM bank.  Bands are host-materialized Hankel tiles; group
  loads alternate between the two HWDGE rings so consecutive groups
  drain concurrently (the straggler group is split in halves across
  both rings for an early pipeline start).  DVE/ACT alternate
  evacuating 2-feature [128, 1024] PSUM chunks to int8; half-group
  output DMAs alternate rings and trail the loads in each FIFO.
"""
import os
import numpy as np
import ml_dtypes

import concourse.bass as bass
import concourse.tile as tile
from concourse import bacc, mybir
from concourse.bass_utils import run_bass_kernel_spmd

B, T, D = 32, 2048, 512
NCORES = 8
DLOC = D // NCORES          # 64 features per core
TB = T // 128               # 16 time blocks
KL, KR = 20, 10
MLO, MHI = -10, 374         # FIR lags m in [MLO, MHI)
GLEN = 511                  # gtab[p] = g[p-137], p in [0,511)
NQ = 3                      # Toeplitz blocks, straggler features
BW = 128 * NQ               # straggler band width
BW2 = 256                   # 2-block band width (light features)
ECOLS = 384                 # host boundary-correction spans outputs t < 384
SH = KR                     # input packed time-shifted by +10 (causal kernel)
G = 8                       # features per group
NG = DLOC // G              # groups per core (group 0 = stragglers)
N = TB * B                  # matmul free dim (tb-major, b-minor)
S = 32.0                    # int8 residual scale (maxint ~99 of 127)
FPB = 2                     # features per PSUM tile (2 banks; bufs=4)

BF = ml_dtypes.bfloat16
F8 = ml_dtypes.float8_e4m3fn

LAST_EXEC_NS = None
LAST_TRACE = None

_nc_cache = None


def _build_tables(l_filter: np.ndarray, r_filter: np.ndarray):
    l = l_filter.astype(np.float64)
    r = r_filter.astype(np.float64)
    H = MHI + 2 * KR        # h needed up to (383 - j + k) <= 393
    h = np.zeros((H, D))
    h[0] = 1.0
    for j in range(1, H):
        kk = min(j, KL - 1)
        h[j] = np.einsum("kd,kd->d", l[1:kk + 1], h[j - kk:j][::-1])
    g = np.zeros((MHI - MLO, D))
    for mi in range(MHI - MLO):
        m = mi + MLO
        acc = np.zeros(D)
        if m >= 0:
            acc = h[m] * (1.0 + l[0])
        for k in range(1, KR + 1):
            if m + k >= 0:
                acc = acc + h[m + k] * r[k - 1]
        g[mi] = acc
    g[KR] -= 1.0            # residual trick: subtract the identity tap (m=0)
    gtab = np.zeros((D, GLEN), dtype=np.float64)
    gtab[:, 137 + MLO:137 + MHI] = g.T * S  # p in [127, 511)
    # host boundary correction (outputs t < ECOLS), contracted with v[0:10]:
    #  + g[t-p]           : head restore (shifted packing drops v[0..9])
    #  - sum_{k>p} h[t-p+k] r[k-1] : FIR over-counts r-taps reading base t'<0
    # (g here already lacks the identity tap, so t<10 rows subtract v[t] too)
    ftab = np.zeros((D, KR, ECOLS), dtype=np.float64)
    tt = np.arange(ECOLS)
    for p in range(KR):
        acc = np.zeros((ECOLS, D))
        m = tt - p
        sel = (m >= MLO) & (m < MHI)
        acc[sel] += g[m[sel] - MLO]
        for k in range(p + 1, KR + 1):
            acc -= h[tt - p + k] * r[k - 1]
        ftab[:, p, :] = acc.T
    return gtab, ftab


def _build_bass():
    nc = bacc.Bacc("TRN2", target_bir_lowering=False, debug=False)
    xin = nc.dram_tensor("xin", [NG, 128, G * N], mybir.dt.float8e4,
                         kind="ExternalInput")
    bd3 = nc.dram_tensor("bd3", [128, G * BW], mybir.dt.bfloat16,
                         kind="ExternalInput")
    bd2 = nc.dram_tensor("bd2", [NG - 1, 128, G * BW2], mybir.dt.bfloat16,
                         kind="ExternalInput")
    ot = nc.dram_tensor("ot", [NG, 128, G * N], mybir.dt.int8,
                        kind="ExternalOutput")
    with tile.TileContext(nc) as tc:
        with tc.tile_pool(name="x", bufs=NG) as xp, \
             tc.tile_pool(name="w", bufs=1) as wp, \
             tc.tile_pool(name="o", bufs=6) as op, \
             tc.tile_pool(name="ps", bufs=4, space="PSUM") as pp:
            xts, bts = [], []
            for gi in range(NG):
                xv = xp.tile([128, G * N], mybir.dt.float8e4)
                bwg = BW if gi == 0 else BW2
                band = wp.tile([128, G * bwg], mybir.dt.bfloat16,
                               tag=f"band{gi}")
                if gi == 0:
                    # split the straggler group across both rings for an
                    # early pipeline start
                    h = G * N // 2
                    nc.sync.dma_start(out=xv[:, 0:h], in_=xin[0][:, 0:h])
                    hb = G * BW // 2
                    nc.scalar.dma_start(out=band[:, 0:hb],
                                        in_=bd3.ap()[:, 0:hb])
                    nc.sync.dma_start(out=xv[:, h:G * N],
                                      in_=xin[0][:, h:G * N])
                    nc.scalar.dma_start(out=band[:, hb:G * BW],
                                        in_=bd3.ap()[:, hb:G * BW])
                else:
                    # xin and band of one group ride OPPOSITE rings so the
                    # group completes at the combined (2-ring) drain rate;
                    # two mid-kernel bands go to the otherwise-idle SWDGE
                    # queue so the HWDGE rings finish before the PE does
                    xeng = nc.sync if gi % 2 == 0 else nc.scalar
                    beng = nc.scalar if gi % 2 == 0 else nc.sync
                    if gi in (4, 6):
                        beng = nc.gpsimd
                    xeng.dma_start(out=xv[:], in_=xin[gi])
                    beng.dma_start(out=band[:], in_=bd2[gi - 1])
                xts.append(xv)
                bts.append(band)

            for gi in range(NG):
                xv, band = xts[gi], bts[gi]
                nq = NQ if gi == 0 else 2
                bwg = BW if gi == 0 else BW2
                ov = op.tile([128, G * N], mybir.dt.int8)
                for fg in range(G // FPB):
                    ps = pp.tile([128, FPB * N], mybir.dt.float32)
                    for ff in range(FPB):
                        f = fg * FPB + ff
                        xo = f * N
                        bo = f * bwg
                        po = ff * N
                        for q in range(nq):
                            nc.tensor.matmul(
                                ps[:, po + 32 * q:po + N],
                                band[:, bo + 128 * q:bo + 128 * (q + 1)],
                                xv[:, xo:xo + N - 32 * q],
                                start=(q == 0), stop=(q == nq - 1))
                    # evacuate the 2-feature [128,1024] tile; DVE/ACT
                    # alternate so neither engine is on every WAR edge.
                    # The very last tile splits across BOTH engines and
                    # the last store across BOTH rings - shortest tail.
                    oo = fg * FPB * N
                    last = (gi == NG - 1 and fg == G // FPB - 1)
                    if last:
                        nc.vector.tensor_copy(ov[:, oo:oo + N], ps[:, 0:N])
                        nc.scalar.copy(ov[:, oo + N:oo + FPB * N],
                                       ps[:, N:FPB * N])
                        lo = oo - FPB * N
                        nc.sync.dma_start(out=ot[gi][:, lo:lo + FPB * N],
                                          in_=ov[:, lo:lo + FPB * N])
                        nc.scalar.dma_start(
                            out=ot[gi][:, oo:oo + FPB * N],
                            in_=ov[:, oo:oo + FPB * N])
                    else:
                        ceng = nc.vector.tensor_copy if fg % 2 == 0 \
                            else nc.scalar.copy
                        ceng(ov[:, oo:oo + FPB * N], ps[:])
                        if fg % 2 == 1:
                            # early-group outputs ride the (otherwise
                            # idle) SWDGE queue; late-group outputs use
                            # the sync ring's post-load window (SP is a
                            # pure dispatcher, nothing to head-block)
                            oeng = nc.gpsimd if gi < 4 else nc.sync
                            oeng.dma_start(
                                out=ot[gi][:, oo - FPB * N:oo + FPB * N],
                                in_=ov[:, oo - FPB * N:oo + FPB * N])
    nc.compile()
    return nc


def kernel(v: np.ndarray, l_filter: np.ndarray, r_filter: np.ndarray) -> np.ndarray:
    global _nc_cache, LAST_EXEC_NS, LAST_TRACE
    v = np.asarray(v, dtype=np.float32)
    gtab, ftab = _build_tables(np.asarray(l_filter), np.asarray(r_filter))

    # pack v: [B,1,T,D] -> per-d tiles [d, a, tb*B + b], partition a = flipped
    # in-block time (t = tb*128 + 127 - a), input pre-shifted: x[t'] = v[t'+SH]
    s = v[:, 0, :, :]                                  # [B, T, D]
    ssh = np.zeros_like(s)
    ssh[:, :T - SH, :] = s[:, SH:, :]
    tiles = ssh.reshape(B, TB, 128, D)                 # [b, tb, i, d]
    xall = tiles.transpose(3, 2, 1, 0)[:, ::-1, :, :]  # [d, a(flip), tb, b]
    xall = np.ascontiguousarray(xall).reshape(D, 128, N).astype(F8)
    # band[d][a, j] = gtab[d, a + j]
    hank3 = np.arange(128)[:, None] + np.arange(BW)[None, :]
    hank2 = hank3[:, :BW2]
    # straggler selection: per-core top-8 FIR-tail norms get the 3rd block
    tails = np.linalg.norm(gtab[:, 137 + 119:], axis=1)

    if _nc_cache is None:
        _nc_cache = _build_bass()
    nc = _nc_cache

    in_maps = []
    perms = []
    for c in range(NCORES):
        d0 = c * DLOC
        tc_ = tails[d0:d0 + DLOC]
        idx = np.argsort(-tc_)
        perm = np.concatenate([idx[:G], np.sort(idx[G:])])  # stragglers first
        perms.append(perm)
        gd = gtab[d0:d0 + DLOC][perm]                  # [DLOC, GLEN] permuted
        xg = xall[d0:d0 + DLOC][perm]                  # [DLOC, 128, N]
        xg = xg.reshape(NG, G, 128, N).transpose(0, 2, 1, 3)
        b3 = gd[:G][:, hank3]                          # [G, 128, BW]
        b2 = gd[G:][:, hank2].reshape(NG - 1, G, 128, BW2)
        in_maps.append({
            "xin": np.ascontiguousarray(xg).reshape(NG, 128, G * N),
            "bd3": np.ascontiguousarray(
                b3.transpose(1, 0, 2)).reshape(128, G * BW).astype(BF),
            "bd2": np.ascontiguousarray(
                b2.transpose(0, 2, 1, 3)).reshape(
                    NG - 1, 128, G * BW2).astype(BF),
        })
    trace = os.environ.get("DFSMN_TRACE", "0") == "1"
    r = run_bass_kernel_spmd(nc, in_maps, list(range(NCORES)), trace=trace)
    LAST_EXEC_NS = r.exec_time_ns
    LAST_TRACE = r.instructions_and_trace
    # unpack: ot [NG, 128, G*N] int8 -> delta/S; out = v + delta + boundary fix
    outs = []
    for c in range(NCORES):
        og = r.results[c]["ot"].reshape(NG, 128, G, N).transpose(0, 2, 1, 3)
        og = og.reshape(DLOC, 128, N)
        inv = np.empty(DLOC, dtype=np.int64)
        inv[perms[c]] = np.arange(DLOC)
        outs.append(og[inv])
    ot_all = np.concatenate(outs, axis=0)              # [D, 128, TB*B] int8

    delta = ot_all.astype(np.float32) * np.float32(1.0 / S)
    delta = delta.reshape(D, 128, TB, B)
    delta = delta.transpose(3, 2, 1, 0).reshape(B, T, D)  # t = tb*128 + i
    out = s + delta
    # exact boundary correction on the host (t < ECOLS only)
    corr = np.einsum("dpt,bpd->btd", ftab, s[:, :KR, :].astype(np.float64))
    out[:, :ECOLS, :] += corr.astype(np.float32)
    return np.ascontiguousarray(out[:, None, :, :])


# revision 23
# speedup vs baseline: 1.0667x; 1.0667x over previous
"""DFSMN forward on 8 Trainium2 NeuronCores (Bass/Tile).

Math: the reference computes
    base_t = (1+l0)*v_t + sum_{k=1..10} r_{k-1} v_{t+k}
    p_t    = base_t + sum_{k=1..19} l_k p_{t-k}        (per-feature IIR)
which equals a per-feature convolution  p = g * v  (plus boundary fixes)
with g built from the recurrence impulse response (geometric decay).

The kernel computes the RESIDUAL delta = p - v on device (the identity
tap is removed from the filter table host-side), scales by S=32 and
emits int8; the host adds v back in fp32.  Start-of-sequence boundary
fixes (head restore of v[0:10], r-tap overcount, identity for t<10)
touch only t < 384 and contract with v[0:10], so they are applied on
the HOST in fp64 - the device runs a pure band-matmul chain.

Mixed FIR length: the recurrence tail beyond lag ~119 is negligible for
~97% of features (median L2 7e-6); only features with near-unstable
recurrences need the third Toeplitz block.  The host sorts each core's
64 features by tail norm: the top-8 form the STRAGGLER group (3 blocks,
lags to i+246); the remaining 56 run 2 blocks (lags to i+118; their
truncated tail is <~1.5e-4, invisible next to the 3.3e-3 int8 floor).
This cuts PE time ~40% and band traffic ~30%.

Device mapping (features sharded 8 ways -> 64 per core):
  per feature: 2-3 Toeplitz-band matmuls (block q covers lags
  [i-137+128q, i-10+128q] for output row i) accumulate 32*delta into
  one PSUM bank.  Bands are host-materialized Hankel tiles; group
  loads alternate between the two HWDGE rings so consecutive groups
  drain concurrently (the straggler group is split in halves across
  both rings for an early pipeline start).  DVE/ACT alternate
  evacuating 2-feature [128, 1024] PSUM chunks to int8; half-group
  output DMAs alternate rings and trail the loads in each FIFO.
"""
import os
import numpy as np
import ml_dtypes

import concourse.bass as bass
import concourse.tile as tile
from concourse import bacc, mybir
from concourse.bass_utils import run_bass_kernel_spmd

B, T, D = 32, 2048, 512
NCORES = 8
DLOC = D // NCORES          # 64 features per core
TB = T // 128               # 16 time blocks
KL, KR = 20, 10
MLO, MHI = -10, 374         # FIR lags m in [MLO, MHI)
GLEN = 511                  # gtab[p] = g[p-137], p in [0,511)
NQ = 3                      # Toeplitz blocks, straggler features
BW = 128 * NQ               # straggler band width
BW2 = 256                   # 2-block band width (light features)
ECOLS = 384                 # host boundary-correction spans outputs t < 384
SH = KR                     # input packed time-shifted by +10 (causal kernel)
G = 8                       # features per group
NG = DLOC // G              # groups per core (group 0 = stragglers)
N = TB * B                  # matmul free dim (tb-major, b-minor)
S = 32.0                    # int8 residual scale (maxint ~99 of 127)
FPB = 2                     # features per PSUM tile (2 banks; bufs=4)

BF = ml_dtypes.bfloat16
F8 = ml_dtypes.float8_e4m3fn

LAST_EXEC_NS = None
LAST_TRACE = None

_nc_cache = None


def _build_tables(l_filter: np.ndarray, r_filter: np.ndarray):
    l = l_filter.astype(np.float64)
    r = r_filter.astype(np.float64)
    H = MHI + 2 * KR        # h needed up to (383 - j + k) <= 393
    h = np.zeros((H, D))
    h[0] = 1.0
    for j in range(1, H):
        kk = min(j, KL - 1)
        h[j] = np.einsum("kd,kd->d", l[1:kk + 1], h[j - kk:j][::-1])
    g = np.zeros((MHI - MLO, D))
    for mi in range(MHI - MLO):
        m = mi + MLO
        acc = np.zeros(D)
        if m >= 0:
            acc = h[m] * (1.0 + l[0])
        for k in range(1, KR + 1):
            if m + k >= 0:
                acc = acc + h[m + k] * r[k - 1]
        g[mi] = acc
    g[KR] -= 1.0            # residual trick: subtract the identity tap (m=0)
    gtab = np.zeros((D, GLEN), dtype=np.float64)
    gtab[:, 137 + MLO:137 + MHI] = g.T * S  # p in [127, 511)
    # host boundary correction (outputs t < ECOLS), contracted with v[0:10]:
    #  + g[t-p]           : head restore (shifted packing drops v[0..9])
    #  - sum_{k>p} h[t-p+k] r[k-1] : FIR over-counts r-taps reading base t'<0
    # (g here already lacks the identity tap, so t<10 rows subtract v[t] too)
    ftab = np.zeros((D, KR, ECOLS), dtype=np.float64)
    tt = np.arange(ECOLS)
    for p in range(KR):
        acc = np.zeros((ECOLS, D))
        m = tt - p
        sel = (m >= MLO) & (m < MHI)
        acc[sel] += g[m[sel] - MLO]
        for k in range(p + 1, KR + 1):
            acc -= h[tt - p + k] * r[k - 1]
        ftab[:, p, :] = acc.T
    return gtab, ftab


def _build_bass():
    nc = bacc.Bacc("TRN2", target_bir_lowering=False, debug=False)
    xin = nc.dram_tensor("xin", [NG, 128, G * N], mybir.dt.float8e4,
                         kind="ExternalInput")
    bd3 = nc.dram_tensor("bd3", [128, G * BW], mybir.dt.bfloat16,
                         kind="ExternalInput")
    bd2 = nc.dram_tensor("bd2", [NG - 1, 128, G * BW2], mybir.dt.bfloat16,
                         kind="ExternalInput")
    ot = nc.dram_tensor("ot", [NG, 128, G * N], mybir.dt.int8,
                        kind="ExternalOutput")
    with tile.TileContext(nc) as tc:
        with tc.tile_pool(name="x", bufs=NG) as xp, \
             tc.tile_pool(name="w", bufs=1) as wp, \
             tc.tile_pool(name="o", bufs=4) as op, \
             tc.tile_pool(name="ps", bufs=4, space="PSUM") as pp:
            xts, bts = [], []
            for gi in range(NG):
                xv = xp.tile([128, G * N], mybir.dt.float8e4)
                bwg = BW if gi == 0 else BW2
                band = wp.tile([128, G * bwg], mybir.dt.bfloat16,
                               tag=f"band{gi}")
                if gi == 0:
                    # split the straggler group across both rings for an
                    # early pipeline start
                    h = G * N // 2
                    nc.sync.dma_start(out=xv[:, 0:h], in_=xin[0][:, 0:h])
                    hb = G * BW // 2
                    nc.scalar.dma_start(out=band[:, 0:hb],
                                        in_=bd3.ap()[:, 0:hb])
                    nc.sync.dma_start(out=xv[:, h:G * N],
                                      in_=xin[0][:, h:G * N])
                    nc.scalar.dma_start(out=band[:, hb:G * BW],
                                        in_=bd3.ap()[:, hb:G * BW])
                else:
                    # xin and band of one group ride OPPOSITE rings so the
                    # group completes at the combined (2-ring) drain rate;
                    # two mid-kernel bands go to the otherwise-idle SWDGE
                    # queue so the HWDGE rings finish before the PE does
                    xeng = nc.sync if gi % 2 == 0 else nc.scalar
                    beng = nc.scalar if gi % 2 == 0 else nc.sync
                    if gi in (4, 6):
                        beng = nc.gpsimd
                    xeng.dma_start(out=xv[:], in_=xin[gi])
                    beng.dma_start(out=band[:], in_=bd2[gi - 1])
                xts.append(xv)
                bts.append(band)

            for gi in range(NG):
                xv, band = xts[gi], bts[gi]
                nq = NQ if gi == 0 else 2
                bwg = BW if gi == 0 else BW2
                ov = op.tile([128, G * N], mybir.dt.int8)
                for fg in range(G // FPB):
                    ps = pp.tile([128, FPB * N], mybir.dt.float32)
                    for ff in range(FPB):
                        f = fg * FPB + ff
                        xo = f * N
                        bo = f * bwg
                        po = ff * N
                        for q in range(nq):
                            nc.tensor.matmul(
                                ps[:, po + 32 * q:po + N],
                                band[:, bo + 128 * q:bo + 128 * (q + 1)],
                                xv[:, xo:xo + N - 32 * q],
                                start=(q == 0), stop=(q == nq - 1))
                    # evacuate the 2-feature [128,1024] tile; DVE/ACT
                    # alternate so neither engine is on every WAR edge
                    oo = fg * FPB * N
                    ceng = nc.vector.tensor_copy if fg % 2 == 0 \
                        else nc.scalar.copy
                    ceng(ov[:, oo:oo + FPB * N], ps[:])
                    if fg % 2 == 1:
                        # early-group outputs ride the (otherwise idle)
                        # SWDGE queue - keeps the HWDGE rings and the ACT
                        # FIFO free of store dispatches; late-group
                        # outputs use the sync ring's post-load window
                        # (SP is a pure dispatcher, nothing to block, and
                        # HWDGE completion avoids the final SWDGE drain)
                        oeng = nc.gpsimd if gi < 4 else nc.sync
                        oeng.dma_start(
                            out=ot[gi][:, oo - FPB * N:oo + FPB * N],
                            in_=ov[:, oo - FPB * N:oo + FPB * N])
    nc.compile()
    return nc


def kernel(v: np.ndarray, l_filter: np.ndarray, r_filter: np.ndarray) -> np.ndarray:
    global _nc_cache, LAST_EXEC_NS, LAST_TRACE
    v = np.asarray(v, dtype=np.float32)
    gtab, ftab = _build_tables(np.asarray(l_filter), np.asarray(r_filter))

    # pack v: [B,1,T,D] -> per-d tiles [d, a, tb*B + b], partition a = flipped
    # in-block time (t = tb*128 + 127 - a), input pre-shifted: x[t'] = v[t'+SH]
    s = v[:, 0, :, :]                                  # [B, T, D]
    ssh = np.zeros_like(s)
    ssh[:, :T - SH, :] = s[:, SH:, :]
    tiles = ssh.reshape(B, TB, 128, D)                 # [b, tb, i, d]
    xall = tiles.transpose(3, 2, 1, 0)[:, ::-1, :, :]  # [d, a(flip), tb, b]
    xall = np.ascontiguousarray(xall).reshape(D, 128, N).astype(F8)
    # band[d][a, j] = gtab[d, a + j]
    hank3 = np.arange(128)[:, None] + np.arange(BW)[None, :]
    hank2 = hank3[:, :BW2]
    # straggler selection: per-core top-8 FIR-tail norms get the 3rd block
    tails = np.linalg.norm(gtab[:, 137 + 119:], axis=1)

    if _nc_cache is None:
        _nc_cache = _build_bass()
    nc = _nc_cache

    in_maps = []
    perms = []
    for c in range(NCORES):
        d0 = c * DLOC
        tc_ = tails[d0:d0 + DLOC]
        idx = np.argsort(-tc_)
        perm = np.concatenate([idx[:G], np.sort(idx[G:])])  # stragglers first
        perms.append(perm)
        gd = gtab[d0:d0 + DLOC][perm]                  # [DLOC, GLEN] permuted
        xg = xall[d0:d0 + DLOC][perm]                  # [DLOC, 128, N]
        xg = xg.reshape(NG, G, 128, N).transpose(0, 2, 1, 3)
        b3 = gd[:G][:, hank3]                          # [G, 128, BW]
        b2 = gd[G:][:, hank2].reshape(NG - 1, G, 128, BW2)
        in_maps.append({
            "xin": np.ascontiguousarray(xg).reshape(NG, 128, G * N),
            "bd3": np.ascontiguousarray(
                b3.transpose(1, 0, 2)).reshape(128, G * BW).astype(BF),
            "bd2": np.ascontiguousarray(
                b2.transpose(0, 2, 1, 3)).reshape(
                    NG - 1, 128, G * BW2).astype(BF),
        })
    trace = os.environ.get("DFSMN_TRACE", "0") == "1"
    r = run_bass_kernel_spmd(nc, in_maps, list(range(NCORES)), trace=trace)
    LAST_EXEC_NS = r.exec_time_ns
    LAST_TRACE = r.instructions_and_trace
    # unpack: ot [NG, 128, G*N] int8 -> delta/S; out = v + delta + boundary fix
    outs = []
    for c in range(NCORES):
        og = r.results[c]["ot"].reshape(NG, 128, G, N).transpose(0, 2, 1, 3)
        og = og.reshape(DLOC, 128, N)
        inv = np.empty(DLOC, dtype=np.int64)
        inv[perms[c]] = np.arange(DLOC)
        outs.append(og[inv])
    ot_all = np.concatenate(outs, axis=0)              # [D, 128, TB*B] int8

    delta = ot_all.astype(np.float32) * np.float32(1.0 / S)
    delta = delta.reshape(D, 128, TB, B)
    delta = delta.transpose(3, 2, 1, 0).reshape(B, T, D)  # t = tb*128 + i
    out = s + delta
    # exact boundary correction on the host (t < ECOLS only)
    corr = np.einsum("dpt,bpd->btd", ftab, s[:, :KR, :].astype(np.float64))
    out[:, :ECOLS, :] += corr.astype(np.float32)
    return np.ascontiguousarray(out[:, None, :, :])
